# revision 1
# baseline (speedup 1.0000x reference)
"""Env-specific MLP heads on 8 trn2 cores.

out[i] = Linear2(relu(Linear1(h[i]))) using the weights of head env_ids[i].

Strategy (expert-parallel with host-side routing):
  - Host sorts tokens by env id. Env e's tokens are split between cores
    2e and 2e+1 (E=4 envs, 8 cores), zero-padded to a common length T.
  - Each core runs a dense 2-layer MLP on its [T, D] token block with a
    single env's weights: no masking, no wasted env compute (the
    reference computes all E envs for every token).
  - Activations live in transposed [feature, token] layout on-chip, so
    both matmuls use natural-layout weight tiles as the stationary
    operand and biases are per-partition ACT-engine bias adds. The host
    pre-transposes h (free) and un-permutes the gathered output (free).
"""

import numpy as np
import ml_dtypes

import concourse.mybir as mybir
import concourse.tile as tile
from concourse import bacc
from concourse.bass_utils import run_bass_kernel_spmd

P = 128
NCORES = 8
NMAX = 512  # one fp32 PSUM bank


def build_nc(T, D=1024, Hdim=2048, A=1024):
    """Bass program for one core: out[A,T] = W2.T@relu(W1.T@xt + b1) + b2."""
    KO1, KO2, AT = D // P, Hdim // P, A // P
    bf16, f32 = mybir.dt.bfloat16, mybir.dt.float32

    nc = bacc.Bacc(
        "TRN2", target_bir_lowering=False, debug=True, num_devices=NCORES
    )

    xt = nc.dram_tensor("xt", [D, T], bf16, kind="ExternalInput")
    w1 = nc.dram_tensor("w1", [D, Hdim], bf16, kind="ExternalInput")
    b1 = nc.dram_tensor("b1", [P, KO2], f32, kind="ExternalInput")
    w2 = nc.dram_tensor("w2", [Hdim, A], bf16, kind="ExternalInput")
    b2 = nc.dram_tensor("b2", [P, AT], f32, kind="ExternalInput")
    out = nc.dram_tensor("out", [A, T], f32, kind="ExternalOutput")

    chunks = []
    t0 = 0
    while t0 < T:
        tn = min(NMAX, T - t0)
        chunks.append((t0, tn))
        t0 += tn

    with tile.TileContext(nc) as tc:
        with (
            tc.tile_pool(name="weights", bufs=1) as wp,
            tc.tile_pool(name="acts", bufs=1) as acts,
            tc.tile_pool(name="ps1", bufs=2, space="PSUM") as pp1,
            tc.tile_pool(name="ps2", bufs=2, space="PSUM") as pp2,
            tc.tile_pool(name="outs", bufs=4) as op,
        ):
            w1_sb = wp.tile([P, KO1, Hdim], bf16, tag="w1")
            w2_sb = wp.tile([P, KO2, A], bf16, tag="w2")
            b1_sb = wp.tile([P, KO2], f32, tag="b1")
            b2_sb = wp.tile([P, AT], f32, tag="b2")
            xt_sb = acts.tile([P, KO1, T], bf16, tag="xt")

            nc.sync.dma_start(b1_sb[:], b1[:])
            nc.sync.dma_start(b2_sb[:], b2[:])
            for k in range(KO1):
                nc.sync.dma_start(xt_sb[:, k], xt[k * P : (k + 1) * P, :])
            for k in range(KO1):
                nc.sync.dma_start(w1_sb[:, k], w1[k * P : (k + 1) * P, :])
            for k in range(KO2):
                nc.sync.dma_start(w2_sb[:, k], w2[k * P : (k + 1) * P, :])

            for t0, tn in chunks:
                # hid^T[H, t0:t0+tn] as KO2 tiles of [128 features, tn]
                hid_sb = acts.tile([P, KO2, tn], bf16, tag=f"hid_{t0}")
                for h in range(KO2):
                    ps = pp1.tile([P, tn], f32, tag="ps1")
                    for k in range(KO1):
                        nc.tensor.matmul(
                            ps[:],
                            w1_sb[:, k, h * P : (h + 1) * P],
                            xt_sb[:, k, t0 : t0 + tn],
                            start=(k == 0),
                            stop=(k == KO1 - 1),
                        )
                    nc.scalar.activation(
                        hid_sb[:, h],
                        ps[:],
                        mybir.ActivationFunctionType.Relu,
                        bias=b1_sb[:, h : h + 1],
                    )
                for a in range(AT):
                    ps = pp2.tile([P, tn], f32, tag="ps2")
                    for k in range(KO2):
                        nc.tensor.matmul(
                            ps[:],
                            w2_sb[:, k, a * P : (a + 1) * P],
                            hid_sb[:, k],
                            start=(k == 0),
                            stop=(k == KO2 - 1),
                        )
                    ot = op.tile([P, tn], f32, tag="ot")
                    nc.scalar.activation(
                        ot[:],
                        ps[:],
                        mybir.ActivationFunctionType.Identity,
                        bias=b2_sb[:, a : a + 1],
                    )
                    nc.sync.dma_start(out[a * P : (a + 1) * P, t0 : t0 + tn], ot[:])

    nc.compile()
    return nc


def make_in_maps(h, env_ids, W1, b1, W2, b2):
    """Route tokens to cores; returns (in_maps, core_tokens, T)."""
    bf16 = ml_dtypes.bfloat16
    B, D = h.shape
    E, _, Hdim = W1.shape
    A = W2.shape[-1]
    cpe = NCORES // E  # cores per env
    assert cpe * E == NCORES

    env = np.asarray(env_ids).reshape(-1).astype(np.int64)
    order = np.argsort(env, kind="stable")
    counts = np.bincount(env, minlength=E)
    starts = np.concatenate([[0], np.cumsum(counts)])
    cap = int(max((counts + cpe - 1) // cpe))
    T = max(-(-cap // 16) * 16, 64)

    in_maps = []
    core_tokens = []
    for e in range(E):
        idx = order[starts[e] : starts[e + 1]]
        parts = np.array_split(idx, cpe)
        w1e = np.ascontiguousarray(W1[e]).astype(bf16)
        w2e = np.ascontiguousarray(W2[e]).astype(bf16)
        b1e = np.ascontiguousarray(
            b1[e].astype(np.float32).reshape(Hdim // P, P).T
        )
        b2e = np.ascontiguousarray(b2[e].astype(np.float32).reshape(A // P, P).T)
        for s in range(cpe):
            tok = parts[s]
            xt = np.zeros((D, T), dtype=bf16)
            if len(tok):
                xt[:, : len(tok)] = h[tok].astype(bf16).T
            in_maps.append({"xt": xt, "w1": w1e, "b1": b1e, "w2": w2e, "b2": b2e})
            core_tokens.append(tok)
    return in_maps, core_tokens, T


def kernel(h, env_ids, W1, b1, W2, b2):
    h = np.asarray(h, dtype=np.float32)
    W1 = np.asarray(W1, dtype=np.float32)
    b1 = np.asarray(b1, dtype=np.float32)
    W2 = np.asarray(W2, dtype=np.float32)
    b2 = np.asarray(b2, dtype=np.float32)

    in_maps, core_tokens, T = make_in_maps(h, env_ids, W1, b1, W2, b2)
    nc = build_nc(T, D=h.shape[1], Hdim=W1.shape[2], A=W2.shape[2])
    res = run_bass_kernel_spmd(nc, in_maps, list(range(NCORES))).results

    B = h.shape[0]
    A = W2.shape[2]
    out = np.zeros((B, A), dtype=np.float32)
    for c in range(NCORES):
        tok = core_tokens[c]
        if len(tok):
            out[tok] = res[c]["out"][:, : len(tok)].T
    return out
